# revision 17
# baseline (speedup 1.0000x reference)
"""DeepSeek-style MLHA (multi-head latent attention) Trainium2 kernel.

Problem shapes (hardcoded):
  x: [B=2, L=2048, D=2048], HEADS=16, HEAD_DIM=128, KV_RANK=256, ATTN=2048.
  q = x @ Wq;  latent = rms_norm(x @ Wkv_down) * kv_norm_w;  kv = latent @ Wkv_up
  k, v = split(kv);  out = softmax_causal(q k^T / sqrt(128)) v;  y = out @ Wo

Sharding: tensor-parallel over heads across 8 cores (2 heads/core).
Each core computes q/k/v for its 2 heads (kv_down replicated), full causal
attention for those heads, and a partial y = attn_out @ Wo[rows of its heads].
Host sums the 8 partial outputs (o_proj input-dim sharding => partial sums).

Device-side layout: everything that feeds the PE array keeps the contraction
dim on partitions. Host passes x pre-transposed (xT [D, B*L]). Projections
produce qT/kT [head_dim, seq] directly; scores are computed transposed
(sT[y,x] = k @ qT) so the softmax normalizer 1/l[x] is applied at the very
end on the o_proj output (x lands on partitions there), and the rms-norm
rstd[y] is applied as a per-partition scale inside the exp() and on v.
All matmuls run in float32r (full PE rate at N>=256). Batches processed
sequentially to halve SBUF residency; weights stay resident across batches.
"""

import numpy as np

import concourse.bacc as bacc
import concourse.mybir as mybir
import concourse.tile as tile
from concourse.bass_utils import run_bass_kernel_spmd

F32 = mybir.dt.float32
F32R = mybir.dt.float32r
AF = mybir.ActivationFunctionType
ALU = mybir.AluOpType

B, L, D = 2, 2048, 2048
BL = B * L                      # 4096
HEADS, HD = 16, 128
R = 256                         # KV_RANK
NC_CORES = 8
H_LOC = HEADS // NC_CORES       # 2 heads per core
DQ = H_LOC * HD                 # 256 local q/k dim
E = H_LOC * HD                  # 256 local v dim
EPS = 1e-6
KC = D // 128                   # 16 contraction chunks over hidden
XC = L // 512                   # 4 phase-1 x chunks per batch
YB = L // 128                   # 16 y chunks per batch
XB = L // 512                   # 4 x blocks per batch


def _emit_batch(tc, b, env):
    nc = tc.nc
    xT, out_d = env["xT"], env["out_d"]
    wq_sb, wkd_sb, wuk_sb, wuv_sb, wo_sb = (
        env["wq_sb"], env["wkd_sb"], env["wuk_sb"], env["wuv_sb"], env["wo_sb"])
    ones1, ones_inv, one33 = env["ones1"], env["ones_inv"], env["one33"]
    zero_b, eps_b, epsh_b = env["zero_b"], env["eps_b"], env["epsh_b"]

    kvp = tc.alloc_tile_pool(name="kvp", bufs=1)
    kT = kvp.tile([128, H_LOC, L], F32R)        # [hd, head, y]
    v_sb = kvp.tile([128, YB, E], F32R)         # [y%128, ychunk, e]
    qtp = tc.alloc_tile_pool(name="qtp", bufs=1)
    qT = qtp.tile([128, H_LOC, L], F32R)        # [hd, head, x]
    rstd_pure = qtp.tile([128, YB], F32)
    rstd_sc = qtp.tile([128, YB], F32)

    # ---- phase 1: qT = Wq^T x^T, latT = Wkd^T x^T, ms = mean(lat^2) ----
    latp = tc.alloc_tile_pool(name="latp", bufs=1)
    latT = latp.tile([128, 2, L], F32R)         # [r%128, r//128, y]
    ms_sb = latp.tile([1, L], F32R)

    xtp = tc.alloc_tile_pool(name="xtp", bufs=20)
    psA = tc.alloc_tile_pool(name="psA", bufs=1, space="PSUM")

    for xc in range(XC):
        sl = slice(xc * 512, (xc + 1) * 512)
        xts = []
        for kc in range(KC):
            xt = xtp.tile([128, 512], F32R, tag="xtk", name=f"xt{xc}_{kc}")
            nc.sync.dma_start(
                out=xt,
                in_=xT[kc * 128:(kc + 1) * 128,
                       b * L + xc * 512:b * L + (xc + 1) * 512])
            xts.append(xt)
        ms_ps = psA.tile([1, 512], F32, tag="ms", bufs=2)
        for oc in range(2):  # q head chunks
            ps = psA.tile([128, 512], F32, tag=f"q{oc}", bufs=2, name=f"q{oc}")
            for kc in range(KC):
                nc.tensor.matmul(ps, lhsT=wq_sb[:, kc, oc * 128:(oc + 1) * 128],
                                 rhs=xts[kc], start=(kc == 0), stop=(kc == KC - 1))
            nc.vector.tensor_copy(qT[:, oc, sl], ps)
        for rc in range(2):  # latent chunks
            ps = psA.tile([128, 512], F32, tag=f"lt{rc}", bufs=1, name=f"lt{rc}")
            for kc in range(KC):
                nc.tensor.matmul(ps, lhsT=wkd_sb[:, kc, rc * 128:(rc + 1) * 128],
                                 rhs=xts[kc], start=(kc == 0), stop=(kc == KC - 1))
            nc.vector.tensor_copy(latT[:, rc, sl], ps)
            sq = xtp.tile([128, 512], F32R, tag="sq", bufs=2)
            nc.scalar.activation(sq, ps, AF.Square, bias=zero_b)
            nc.tensor.matmul(ms_ps, lhsT=ones_inv, rhs=sq,
                             start=(rc == 0), stop=(rc == 1), skip_group_check=True)
        nc.scalar.copy(ms_sb[:, sl], ms_ps)

    # rstd: transpose ms [1, L] -> msT [128, YB] via K=1 outer-product matmuls
    msT_ps = psA.tile([128, YB], F32, tag="ms", bufs=2)
    for j in range(YB):
        nc.tensor.matmul(msT_ps[:, j:j + 1],
                         lhsT=ms_sb[:, j * 128:(j + 1) * 128].bitcast(F32),
                         rhs=one33[0:1].bitcast(F32), start=True, stop=True,
                         skip_group_check=True)
    t_p = xtp.tile([128, YB], F32, tag="tp", bufs=1)
    nc.scalar.activation(t_p, msT_ps, AF.Sqrt, bias=eps_b, scale=1.0)
    nc.vector.reciprocal(rstd_pure, t_p)
    t_s = xtp.tile([128, YB], F32, tag="ts", bufs=1)
    nc.scalar.activation(t_s, msT_ps, AF.Sqrt, bias=epsh_b, scale=float(HD))
    nc.vector.reciprocal(rstd_sc, t_s)

    xtp.release()

    # ---- phase 2: kT = Wuk^T latT, v = lat @ Wuv (rstd folded into v) ----
    for yc2 in range(XC):  # 512-wide y chunks
        sl = slice(yc2 * 512, (yc2 + 1) * 512)
        for ec in range(H_LOC):
            ps = psA.tile([128, 512], F32, tag=f"q{ec}", bufs=2, name=f"kv{ec}")
            for rc in range(2):
                nc.tensor.matmul(ps, lhsT=wuk_sb[:, rc, ec * 128:(ec + 1) * 128],
                                 rhs=latT[:, rc, sl], start=(rc == 0), stop=(rc == 1))
            nc.vector.tensor_copy(kT[:, ec, sl], ps)
        for j in range(4):
            yg = yc2 * 4 + j
            ps = psA.tile([128, E], F32, tag="lt0", bufs=1, name="vps")
            for rc in range(2):
                nc.tensor.matmul(ps, lhsT=latT[:, rc, yg * 128:(yg + 1) * 128],
                                 rhs=wuv_sb[:, rc, :], start=(rc == 0), stop=(rc == 1))
            nc.vector.tensor_scalar_mul(v_sb[:, yg, :], ps, rstd_pure[:, yg:yg + 1])

    psA.release()
    latp.release()

    # ---- attention + o_proj ----
    workp = tc.alloc_tile_pool(name="workp", bufs=3)
    psC = tc.alloc_tile_pool(name="psC", bufs=1, space="PSUM")

    for xb in range(XB):
        xsl = slice(xb * 512, (xb + 1) * 512)
        nyc = 4 * xb + 4               # causal: y chunks needed
        ao_ps = [psC.tile([128, 512], F32, tag=f"ao{h}", bufs=1, name=f"ao{h}")
                 for h in range(H_LOC)]
        l_ps = psC.tile([1, 2 * 512], F32, tag="lmix", bufs=1)
        for iy in range(nyc):
            st = psC.tile([128, 2 * 512], F32, tag="st", bufs=1)
            at = workp.tile([128, 2 * 512], F32R, tag="at", bufs=4)
            for h in range(H_LOC):
                nc.tensor.matmul(st[:, h * 512:(h + 1) * 512],
                                 lhsT=kT[:, h, iy * 128:(iy + 1) * 128],
                                 rhs=qT[:, h, xsl], start=True, stop=True,
                                 skip_group_check=True)
            nc.scalar.activation(at, st, AF.Exp, bias=zero_b,
                                 scale=rstd_sc[:, iy:iy + 1])
            if iy >= 4 * xb:           # diagonal chunk: zero out y > x
                off = (iy - 4 * xb) * 128
                for h in range(H_LOC):
                    nc.gpsimd.affine_select(
                        out=at[:, h * 512:(h + 1) * 512],
                        in_=at[:, h * 512:(h + 1) * 512],
                        compare_op=ALU.is_ge, fill=0.0, base=-off,
                        pattern=[[1, 512]], channel_multiplier=-1)
            for h in range(H_LOC):
                nc.tensor.matmul(ao_ps[h],
                                 lhsT=v_sb[:, iy, h * 128:(h + 1) * 128],
                                 rhs=at[:, h * 512:(h + 1) * 512],
                                 start=(iy == 0), stop=(iy == nyc - 1),
                                 skip_group_check=True)
                nc.tensor.matmul(l_ps[0:1, h * 512:(h + 1) * 512],
                                 lhsT=ones1, rhs=at[:, h * 512:(h + 1) * 512],
                                 start=(iy == 0), stop=(iy == nyc - 1),
                                 skip_group_check=True)
        ao_sb = workp.tile([128, H_LOC, 512], F32R, tag="ao_sb", bufs=2)
        for h in range(H_LOC):
            nc.vector.tensor_copy(ao_sb[:, h, :], ao_ps[h])
        l_sb = workp.tile([1, 2 * 512], F32R, tag="l_sb", bufs=2)
        nc.scalar.copy(l_sb, l_ps)
        lT_ps = psC.tile([128, 2 * 4], F32, tag="lmix", bufs=1)
        for h in range(H_LOC):
            for xs in range(4):
                nc.tensor.matmul(
                    lT_ps[:, h * 4 + xs:h * 4 + xs + 1],
                    lhsT=l_sb[0:1, h * 512 + xs * 128:h * 512 + (xs + 1) * 128]
                    .bitcast(F32),
                    rhs=one33[0:1].bitcast(F32), start=True, stop=True,
                    skip_group_check=True)
        recip_l = workp.tile([128, 2 * 4], F32, tag="recip", bufs=2)
        nc.vector.reciprocal(recip_l, lT_ps)

        for xs in range(4):
            r0 = b * L + xb * 512 + xs * 128
            for mc in range(4):
                msl = slice(mc * 512, (mc + 1) * 512)
                wps = [psC.tile([128, 512], F32, tag=f"w{h}", bufs=1, name=f"w{h}")
                       for h in range(H_LOC)]
                for h in range(H_LOC):
                    nc.tensor.matmul(wps[h],
                                     lhsT=ao_sb[:, h, xs * 128:(xs + 1) * 128],
                                     rhs=wo_sb[:, h, msl], start=True, stop=True,
                                     skip_group_check=True)
                t = workp.tile([128, 512], F32, tag="wt")
                nc.scalar.activation(t, wps[0], AF.Copy, scale=recip_l[:, xs:xs + 1])
                o = workp.tile([128, 512], F32, tag="o")
                nc.vector.scalar_tensor_tensor(
                    out=o, in0=wps[1], scalar=recip_l[:, 4 + xs:4 + xs + 1],
                    in1=t, op0=ALU.mult, op1=ALU.add)
                nc.gpsimd.dma_start(out=out_d[r0:r0 + 128, msl], in_=o)

    psC.release()
    workp.release()
    qtp.release()
    kvp.release()


def _emit(tc):
    nc = tc.nc
    env = {}
    env["xT"] = nc.dram_tensor("xT", [D, BL], F32R, kind="ExternalInput").ap()
    wq_d = nc.dram_tensor("wq", [D, DQ], F32R, kind="ExternalInput").ap()
    wkd_d = nc.dram_tensor("wkd", [D, R], F32R, kind="ExternalInput").ap()
    wuk_d = nc.dram_tensor("wuk", [R, DQ], F32R, kind="ExternalInput").ap()
    wuv_d = nc.dram_tensor("wuv", [R, E], F32R, kind="ExternalInput").ap()
    wo_d = nc.dram_tensor("wo", [E, D], F32R, kind="ExternalInput").ap()
    env["out_d"] = nc.dram_tensor("out", [BL, D], F32, kind="ExternalOutput").ap()

    constp = tc.alloc_tile_pool(name="constp", bufs=1)
    tmp1 = constp.tile([128, 1], F32)
    nc.vector.memset(tmp1, 1.0)
    tmp2 = constp.tile([128, 1], F32)
    nc.vector.memset(tmp2, 1.0 / R)
    env["ones1"] = constp.tile([128, 1], F32R, name="ones1")
    nc.scalar.copy(env["ones1"], tmp1)
    env["ones_inv"] = constp.tile([128, 1], F32R, name="ones_inv")
    nc.scalar.copy(env["ones_inv"], tmp2)
    tmp3 = constp.tile([33, 1], F32)
    nc.vector.memset(tmp3, 1.0)
    env["one33"] = constp.tile([33, 1], F32R, name="one33")
    nc.scalar.copy(env["one33"], tmp3)
    env["zero_b"] = constp.tile([128, 1], F32, name="zero_b")
    nc.vector.memset(env["zero_b"], 0.0)
    env["eps_b"] = constp.tile([128, 1], F32, name="eps_b")
    nc.vector.memset(env["eps_b"], EPS)
    env["epsh_b"] = constp.tile([128, 1], F32, name="epsh_b")
    nc.vector.memset(env["epsh_b"], HD * EPS)

    wp = tc.alloc_tile_pool(name="wp", bufs=1)
    env["wq_sb"] = wq_sb = wp.tile([128, KC, DQ], F32R, name="wq_sb")
    env["wkd_sb"] = wkd_sb = wp.tile([128, KC, R], F32R, name="wkd_sb")
    for kc in range(KC):
        nc.scalar.dma_start(out=wq_sb[:, kc, :], in_=wq_d[kc * 128:(kc + 1) * 128, :])
        nc.scalar.dma_start(out=wkd_sb[:, kc, :], in_=wkd_d[kc * 128:(kc + 1) * 128, :])
    env["wuk_sb"] = wuk_sb = wp.tile([128, 2, DQ], F32R, name="wuk_sb")
    env["wuv_sb"] = wuv_sb = wp.tile([128, 2, E], F32R, name="wuv_sb")
    nc.scalar.dma_start(out=wuk_sb, in_=wuk_d.rearrange("(c p) n -> p c n", p=128))
    nc.scalar.dma_start(out=wuv_sb, in_=wuv_d.rearrange("(c p) n -> p c n", p=128))
    env["wo_sb"] = wo_sb = wp.tile([128, H_LOC, D], F32R, name="wo_sb")
    for kc in range(H_LOC):
        nc.scalar.dma_start(out=wo_sb[:, kc, :], in_=wo_d[kc * 128:(kc + 1) * 128, :])

    for b in range(B):
        _emit_batch(tc, b, env)

    wp.release()
    constp.release()


_NC_CACHE = None


def _build():
    global _NC_CACHE
    if _NC_CACHE is None:
        nc = bacc.Bacc()
        with tile.TileContext(nc) as tc:
            _emit(tc)
        nc.compile()
        _NC_CACHE = nc
    return _NC_CACHE


def make_in_maps(inputs):
    x = np.asarray(inputs["x"], dtype=np.float32)
    xT = np.ascontiguousarray(x.reshape(BL, D).T)
    Wq = np.asarray(inputs["Wq"], dtype=np.float32)
    Wkd = np.ascontiguousarray(np.asarray(inputs["Wkv_down"], dtype=np.float32))
    Wup = np.asarray(inputs["Wkv_up"], dtype=np.float32) * np.asarray(
        inputs["kv_norm_w"], dtype=np.float32)[:, None]
    Wo = np.asarray(inputs["Wo"], dtype=np.float32)

    in_maps = []
    for c in range(NC_CORES):
        in_maps.append({
            "xT": xT,
            "wq": np.ascontiguousarray(Wq[:, c * DQ:(c + 1) * DQ]),
            "wkd": Wkd,
            "wuk": np.ascontiguousarray(Wup[:, c * DQ:(c + 1) * DQ]),
            "wuv": np.ascontiguousarray(
                Wup[:, HEADS * HD + c * E:HEADS * HD + (c + 1) * E]),
            "wo": np.ascontiguousarray(Wo[c * E:(c + 1) * E, :]),
        })
    return in_maps


def kernel(x, Wq, Wkv_down, kv_norm_w, Wkv_up, Wo):
    in_maps = make_in_maps(dict(x=x, Wq=Wq, Wkv_down=Wkv_down,
                                kv_norm_w=kv_norm_w, Wkv_up=Wkv_up, Wo=Wo))
    nc = _build()
    res = run_bass_kernel_spmd(nc, in_maps, core_ids=list(range(NC_CORES)))
    acc = res.results[0]["out"].astype(np.float32)
    for r in res.results[1:]:
        acc = acc + r["out"]
    return acc.reshape(B, L, D)


# revision 32
# speedup vs baseline: 1.0368x; 1.0368x over previous
"""DeepSeek-style MLHA (multi-head latent attention) Trainium2 kernel.

Problem shapes (hardcoded):
  x: [B=2, L=2048, D=2048], HEADS=16, HEAD_DIM=128, KV_RANK=256, ATTN=2048.
  q = x @ Wq;  latent = rms_norm(x @ Wkv_down) * kv_norm_w;  kv = latent @ Wkv_up
  k, v = split(kv);  out = softmax_causal(q k^T / sqrt(128)) v;  y = out @ Wo

Sharding: tensor-parallel over heads across 8 cores (2 heads/core).
Each core computes q/k/v for its 2 heads (kv_down replicated), full causal
attention for those heads, and a partial y = attn_out @ Wo[rows of its heads].
Host sums the 8 partial outputs (o_proj input-dim sharding => partial sums).

Device-side layout: everything that feeds the PE array keeps the contraction
dim on partitions. Host passes x pre-transposed (xT [D, B*L]). Projections
produce qT/kT [head_dim, seq] directly; scores are computed transposed
(sT[y,x] = k @ qT) so the softmax normalizer 1/l[x] is applied at the very
end on the o_proj output (x lands on partitions there), and the rms-norm
rstd[y] is applied as a per-partition scale inside the exp() and on v.
All matmuls run in float32r (full PE rate at N>=256). Batches processed
sequentially to halve SBUF residency; weights stay resident across batches.
"""

import numpy as np

import concourse.bacc as bacc
import concourse.mybir as mybir
import concourse.tile as tile
from concourse.bass_utils import run_bass_kernel_spmd

F32 = mybir.dt.float32
F32R = mybir.dt.float32r
AF = mybir.ActivationFunctionType
ALU = mybir.AluOpType

B, L, D = 2, 2048, 2048
BL = B * L                      # 4096
HEADS, HD = 16, 128
R = 256                         # KV_RANK
NC_CORES = 8
H_LOC = HEADS // NC_CORES       # 2 heads per core
DQ = H_LOC * HD                 # 256 local q/k dim
E = H_LOC * HD                  # 256 local v dim
EPS = 1e-6
KC = D // 128                   # 16 contraction chunks over hidden
XC = L // 512                   # 4 phase-1 x chunks per batch
YB = L // 128                   # 16 y chunks per batch
XB = L // 512                   # 4 x blocks per batch


def _emit_batch(tc, b, env):
    nc = tc.nc
    xT, out_d = env["xT"], env["out_d"]
    wq_sb, wkd_sb, wuk_sb, wuv_sb, wo_sb = (
        env["wq_sb"], env["wkd_sb"], env["wuk_sb"], env["wuv_sb"], env["wo_sb"])
    ones1, ones_inv, one33 = env["ones1"], env["ones_inv"], env["one33"]
    zero_b, eps_b, epsh_b = env["zero_b"], env["eps_b"], env["epsh_b"]

    kvp = tc.alloc_tile_pool(name="kvp", bufs=1)
    kT = kvp.tile([128, H_LOC, L], F32R)        # [hd, head, y]
    v_sb = kvp.tile([128, YB, E], F32R)         # [y%128, ychunk, e]
    qtp = tc.alloc_tile_pool(name="qtp", bufs=1)
    qT = qtp.tile([128, H_LOC, L], F32R)        # [hd, head, x]
    rstd_pure = qtp.tile([128, YB], F32)
    rstd_sc = qtp.tile([128, YB], F32)

    # ---- phase 1: qT = Wq^T x^T, latT = Wkd^T x^T, ms = mean(lat^2) ----
    latp = tc.alloc_tile_pool(name="latp", bufs=1)
    latT = latp.tile([128, 2, L], F32R)         # [r%128, r//128, y]
    ms_sb = latp.tile([1, L], F32R)

    xtp = tc.alloc_tile_pool(name="xtp", bufs=20)
    psA = tc.alloc_tile_pool(name="psA", bufs=1, space="PSUM")

    for xc in range(XC):
        sl = slice(xc * 512, (xc + 1) * 512)
        xts = []
        for kc in range(KC):
            xt = xtp.tile([128, 512], F32R, tag="xtk", name=f"xt{xc}_{kc}")
            nc.sync.dma_start(
                out=xt,
                in_=xT[kc * 128:(kc + 1) * 128,
                       b * L + xc * 512:b * L + (xc + 1) * 512])
            xts.append(xt)
        ms_ps = psA.tile([1, 512], F32, tag="ms", bufs=1)
        for oc in range(2):  # q head chunks
            ps = psA.tile([128, 512], F32, tag=f"q{oc}", bufs=2 - oc, name=f"q{oc}")
            for kc in range(KC):
                nc.tensor.matmul(ps, lhsT=wq_sb[:, kc, oc * 128:(oc + 1) * 128],
                                 rhs=xts[kc], start=(kc == 0), stop=(kc == KC - 1))
            nc.vector.tensor_copy(qT[:, oc, sl], ps)
        for rc in range(2):  # latent chunks
            ps = psA.tile([128, 512], F32, tag=f"lt{rc}", bufs=2, name=f"lt{rc}")
            for kc in range(KC):
                nc.tensor.matmul(ps, lhsT=wkd_sb[:, kc, rc * 128:(rc + 1) * 128],
                                 rhs=xts[kc], start=(kc == 0), stop=(kc == KC - 1))
            nc.vector.tensor_copy(latT[:, rc, sl], ps)
            sq = xtp.tile([128, 512], F32R, tag="sq", bufs=2)
            nc.scalar.activation(sq, ps, AF.Square, bias=zero_b)
            nc.tensor.matmul(ms_ps, lhsT=ones_inv, rhs=sq,
                             start=(rc == 0), stop=(rc == 1), skip_group_check=True)
        nc.scalar.copy(ms_sb[:, sl], ms_ps)

    # rstd: transpose ms [1, L] -> msT [128, YB] via K=1 outer-product matmuls
    msT_ps = psA.tile([128, YB], F32, tag="ms", bufs=1)
    for j in range(YB):
        nc.tensor.matmul(msT_ps[:, j:j + 1],
                         lhsT=ms_sb[:, j * 128:(j + 1) * 128].bitcast(F32),
                         rhs=one33[0:1].bitcast(F32), start=True, stop=True,
                         skip_group_check=True)
    t_p = xtp.tile([128, YB], F32, tag="tp", bufs=1)
    nc.scalar.activation(t_p, msT_ps, AF.Sqrt, bias=eps_b, scale=1.0)
    nc.vector.reciprocal(rstd_pure, t_p)
    t_s = xtp.tile([128, YB], F32, tag="ts", bufs=1)
    nc.scalar.activation(t_s, msT_ps, AF.Sqrt, bias=epsh_b, scale=float(HD))
    nc.vector.reciprocal(rstd_sc, t_s)

    xtp.release()

    # ---- phase 2: kT = Wuk^T latT, v = lat @ Wuv (rstd folded into v) ----
    for yc2 in range(XC):  # 512-wide y chunks
        sl = slice(yc2 * 512, (yc2 + 1) * 512)
        for ec in range(H_LOC):
            ps = psA.tile([128, 512], F32, tag=f"q{ec}", bufs=2 - ec, name=f"kv{ec}")
            for rc in range(2):
                nc.tensor.matmul(ps, lhsT=wuk_sb[:, rc, ec * 128:(ec + 1) * 128],
                                 rhs=latT[:, rc, sl], start=(rc == 0), stop=(rc == 1))
            nc.scalar.copy(kT[:, ec, sl], ps)
        for j in range(4):
            yg = yc2 * 4 + j
            ps = psA.tile([128, E], F32, tag="lt0", bufs=2, name="vps")
            for rc in range(2):
                nc.tensor.matmul(ps, lhsT=latT[:, rc, yg * 128:(yg + 1) * 128],
                                 rhs=wuv_sb[:, rc, :], start=(rc == 0), stop=(rc == 1))
            nc.vector.tensor_scalar_mul(v_sb[:, yg, :], ps, rstd_pure[:, yg:yg + 1])

    psA.release()
    latp.release()

    # ---- attention + o_proj ----
    workp = tc.alloc_tile_pool(name="workp", bufs=3)
    psC = tc.alloc_tile_pool(name="psC", bufs=1, space="PSUM")

    for xb in range(XB):
        xsl = slice(xb * 512, (xb + 1) * 512)
        nyc = 4 * xb + 4               # causal: y chunks needed
        ao_ps = [psC.tile([128, 512], F32, tag=f"ao{h}", bufs=1, name=f"ao{h}")
                 for h in range(H_LOC)]
        l_ps = psC.tile([1, 2 * 512], F32, tag="lmix", bufs=1)
        for iy in range(nyc):
            st = psC.tile([128, 2 * 512], F32, tag="st", bufs=1)
            at = workp.tile([128, 2 * 512], F32R, tag="at", bufs=6)
            for h in range(H_LOC):
                nc.tensor.matmul(st[:, h * 512:(h + 1) * 512],
                                 lhsT=kT[:, h, iy * 128:(iy + 1) * 128],
                                 rhs=qT[:, h, xsl], start=True, stop=True,
                                 skip_group_check=True)
            nc.scalar.activation(at, st, AF.Exp, bias=zero_b,
                                 scale=rstd_sc[:, iy:iy + 1])
            if iy >= 4 * xb:           # diagonal chunk: zero out y > x
                off = (iy - 4 * xb) * 128
                for h in range(H_LOC):
                    nc.gpsimd.affine_select(
                        out=at[:, h * 512:(h + 1) * 512],
                        in_=at[:, h * 512:(h + 1) * 512],
                        compare_op=ALU.is_ge, fill=0.0, base=-off,
                        pattern=[[1, 512]], channel_multiplier=-1)
            for h in range(H_LOC):
                nc.tensor.matmul(ao_ps[h],
                                 lhsT=v_sb[:, iy, h * 128:(h + 1) * 128],
                                 rhs=at[:, h * 512:(h + 1) * 512],
                                 start=(iy == 0), stop=(iy == nyc - 1),
                                 skip_group_check=True)
                nc.tensor.matmul(l_ps[0:1, h * 512:(h + 1) * 512],
                                 lhsT=ones1, rhs=at[:, h * 512:(h + 1) * 512],
                                 start=(iy == 0), stop=(iy == nyc - 1),
                                 skip_group_check=True)
        ao_sb = workp.tile([128, H_LOC, 512], F32R, tag="ao_sb", bufs=2)
        for h in range(H_LOC):
            nc.vector.tensor_copy(ao_sb[:, h, :], ao_ps[h])
        l_sb = workp.tile([1, 2 * 512], F32R, tag="l_sb", bufs=2)
        nc.scalar.copy(l_sb, l_ps)
        lT_ps = psC.tile([128, 2 * 4], F32, tag="lmix", bufs=1)
        for h in range(H_LOC):
            for xs in range(4):
                nc.tensor.matmul(
                    lT_ps[:, h * 4 + xs:h * 4 + xs + 1],
                    lhsT=l_sb[0:1, h * 512 + xs * 128:h * 512 + (xs + 1) * 128]
                    .bitcast(F32),
                    rhs=one33[0:1].bitcast(F32), start=True, stop=True,
                    skip_group_check=True)
        recip_l = workp.tile([128, 2 * 4], F32, tag="recip", bufs=2)
        nc.vector.reciprocal(recip_l, lT_ps)

        for xs in range(4):
            r0 = b * L + xb * 512 + xs * 128
            for mc in range(4):
                msl = slice(mc * 512, (mc + 1) * 512)
                wps = [psC.tile([128, 512], F32, tag=f"w{h}", bufs=1, name=f"w{h}")
                       for h in range(H_LOC)]
                for h in range(H_LOC):
                    nc.tensor.matmul(wps[h],
                                     lhsT=ao_sb[:, h, xs * 128:(xs + 1) * 128],
                                     rhs=wo_sb[:, h, msl], start=True, stop=True,
                                     skip_group_check=True)
                t = workp.tile([128, 512], F32, tag="wt")
                nc.scalar.activation(t, wps[0], AF.Copy, scale=recip_l[:, xs:xs + 1])
                o = workp.tile([128, 512], F32, tag="o")
                nc.vector.scalar_tensor_tensor(
                    out=o, in0=wps[1], scalar=recip_l[:, 4 + xs:4 + xs + 1],
                    in1=t, op0=ALU.mult, op1=ALU.add)
                nc.gpsimd.dma_start(out=out_d[r0:r0 + 128, msl], in_=o)

    psC.release()
    workp.release()
    qtp.release()
    kvp.release()


def _emit(tc):
    nc = tc.nc
    env = {}
    env["xT"] = nc.dram_tensor("xT", [D, BL], F32R, kind="ExternalInput").ap()
    wq_d = nc.dram_tensor("wq", [D, DQ], F32R, kind="ExternalInput").ap()
    wkd_d = nc.dram_tensor("wkd", [D, R], F32R, kind="ExternalInput").ap()
    wuk_d = nc.dram_tensor("wuk", [R, DQ], F32R, kind="ExternalInput").ap()
    wuv_d = nc.dram_tensor("wuv", [R, E], F32R, kind="ExternalInput").ap()
    wo_d = nc.dram_tensor("wo", [E, D], F32R, kind="ExternalInput").ap()
    env["out_d"] = nc.dram_tensor("out", [BL, D], F32, kind="ExternalOutput").ap()

    constp = tc.alloc_tile_pool(name="constp", bufs=1)
    tmp1 = constp.tile([128, 1], F32)
    nc.vector.memset(tmp1, 1.0)
    tmp2 = constp.tile([128, 1], F32)
    nc.vector.memset(tmp2, 1.0 / R)
    env["ones1"] = constp.tile([128, 1], F32R, name="ones1")
    nc.scalar.copy(env["ones1"], tmp1)
    env["ones_inv"] = constp.tile([128, 1], F32R, name="ones_inv")
    nc.scalar.copy(env["ones_inv"], tmp2)
    tmp3 = constp.tile([33, 1], F32)
    nc.vector.memset(tmp3, 1.0)
    env["one33"] = constp.tile([33, 1], F32R, name="one33")
    nc.scalar.copy(env["one33"], tmp3)
    env["zero_b"] = constp.tile([128, 1], F32, name="zero_b")
    nc.vector.memset(env["zero_b"], 0.0)
    env["eps_b"] = constp.tile([128, 1], F32, name="eps_b")
    nc.vector.memset(env["eps_b"], EPS)
    env["epsh_b"] = constp.tile([128, 1], F32, name="epsh_b")
    nc.vector.memset(env["epsh_b"], HD * EPS)

    wp = tc.alloc_tile_pool(name="wp", bufs=1)
    env["wq_sb"] = wq_sb = wp.tile([128, KC, DQ], F32R, name="wq_sb")
    env["wkd_sb"] = wkd_sb = wp.tile([128, KC, R], F32R, name="wkd_sb")
    for kc in range(KC):
        nc.scalar.dma_start(out=wq_sb[:, kc, :], in_=wq_d[kc * 128:(kc + 1) * 128, :])
        nc.scalar.dma_start(out=wkd_sb[:, kc, :], in_=wkd_d[kc * 128:(kc + 1) * 128, :])
    env["wuk_sb"] = wuk_sb = wp.tile([128, 2, DQ], F32R, name="wuk_sb")
    env["wuv_sb"] = wuv_sb = wp.tile([128, 2, E], F32R, name="wuv_sb")
    nc.scalar.dma_start(out=wuk_sb, in_=wuk_d.rearrange("(c p) n -> p c n", p=128))
    nc.scalar.dma_start(out=wuv_sb, in_=wuv_d.rearrange("(c p) n -> p c n", p=128))
    env["wo_sb"] = wo_sb = wp.tile([128, H_LOC, D], F32R, name="wo_sb")
    for kc in range(H_LOC):
        nc.scalar.dma_start(out=wo_sb[:, kc, :], in_=wo_d[kc * 128:(kc + 1) * 128, :])

    for b in range(B):
        _emit_batch(tc, b, env)

    wp.release()
    constp.release()


_NC_CACHE = None


def _build():
    global _NC_CACHE
    if _NC_CACHE is None:
        nc = bacc.Bacc()
        with tile.TileContext(nc) as tc:
            _emit(tc)
        nc.compile()
        _NC_CACHE = nc
    return _NC_CACHE


def make_in_maps(inputs):
    x = np.asarray(inputs["x"], dtype=np.float32)
    xT = np.ascontiguousarray(x.reshape(BL, D).T)
    Wq = np.asarray(inputs["Wq"], dtype=np.float32)
    Wkd = np.ascontiguousarray(np.asarray(inputs["Wkv_down"], dtype=np.float32))
    Wup = np.asarray(inputs["Wkv_up"], dtype=np.float32) * np.asarray(
        inputs["kv_norm_w"], dtype=np.float32)[:, None]
    Wo = np.asarray(inputs["Wo"], dtype=np.float32)

    in_maps = []
    for c in range(NC_CORES):
        in_maps.append({
            "xT": xT,
            "wq": np.ascontiguousarray(Wq[:, c * DQ:(c + 1) * DQ]),
            "wkd": Wkd,
            "wuk": np.ascontiguousarray(Wup[:, c * DQ:(c + 1) * DQ]),
            "wuv": np.ascontiguousarray(
                Wup[:, HEADS * HD + c * E:HEADS * HD + (c + 1) * E]),
            "wo": np.ascontiguousarray(Wo[c * E:(c + 1) * E, :]),
        })
    return in_maps


def kernel(x, Wq, Wkv_down, kv_norm_w, Wkv_up, Wo):
    in_maps = make_in_maps(dict(x=x, Wq=Wq, Wkv_down=Wkv_down,
                                kv_norm_w=kv_norm_w, Wkv_up=Wkv_up, Wo=Wo))
    nc = _build()
    res = run_bass_kernel_spmd(nc, in_maps, core_ids=list(range(NC_CORES)))
    acc = res.results[0]["out"].astype(np.float32)
    for r in res.results[1:]:
        acc = acc + r["out"]
    return acc.reshape(B, L, D)


# revision 35
# speedup vs baseline: 1.0532x; 1.0158x over previous
"""DeepSeek-style MLHA (multi-head latent attention) Trainium2 kernel.

Problem shapes (hardcoded):
  x: [B=2, L=2048, D=2048], HEADS=16, HEAD_DIM=128, KV_RANK=256, ATTN=2048.
  q = x @ Wq;  latent = rms_norm(x @ Wkv_down) * kv_norm_w;  kv = latent @ Wkv_up
  k, v = split(kv);  out = softmax_causal(q k^T / sqrt(128)) v;  y = out @ Wo

Sharding: tensor-parallel over heads across 8 cores (2 heads/core).
Each core computes q/k/v for its 2 heads (kv_down replicated), full causal
attention for those heads, and a partial y = attn_out @ Wo[rows of its heads].
Host sums the 8 partial outputs (o_proj input-dim sharding => partial sums).

Device-side layout: everything that feeds the PE array keeps the contraction
dim on partitions. Host passes x pre-transposed (xT [D, B*L]). Projections
produce qT/kT [head_dim, seq] directly; scores are computed transposed
(sT[y,x] = k @ qT) so the softmax normalizer 1/l[x] is applied at the very
end on the o_proj output (x lands on partitions there), and the rms-norm
rstd[y] is applied as a per-partition scale inside the exp() and on v.
All matmuls run in float32r (full PE rate at N>=256). Batches processed
sequentially to halve SBUF residency; weights stay resident across batches.
"""

import numpy as np

import concourse.bacc as bacc
import concourse.mybir as mybir
import concourse.tile as tile
from concourse.bass_utils import run_bass_kernel_spmd

F32 = mybir.dt.float32
F32R = mybir.dt.float32r
AF = mybir.ActivationFunctionType
ALU = mybir.AluOpType

B, L, D = 2, 2048, 2048
BL = B * L                      # 4096
HEADS, HD = 16, 128
R = 256                         # KV_RANK
NC_CORES = 8
H_LOC = HEADS // NC_CORES       # 2 heads per core
DQ = H_LOC * HD                 # 256 local q/k dim
E = H_LOC * HD                  # 256 local v dim
EPS = 1e-6
KC = D // 128                   # 16 contraction chunks over hidden
XC = L // 512                   # 4 phase-1 x chunks per batch
YB = L // 128                   # 16 y chunks per batch
XB = L // 512                   # 4 x blocks per batch


def _emit_batch(tc, b, env):
    nc = tc.nc
    xT, out_d = env["xT"], env["out_d"]
    wq_sb, wkd_sb, wuk_sb, wuv_sb, wo_sb = (
        env["wq_sb"], env["wkd_sb"], env["wuk_sb"], env["wuv_sb"], env["wo_sb"])
    ones1, ones_inv, one33 = env["ones1"], env["ones_inv"], env["one33"]
    zero_b, eps_b, epsh_b = env["zero_b"], env["eps_b"], env["epsh_b"]

    xtp = tc.alloc_tile_pool(name="xtp", bufs=20)
    kvp = tc.alloc_tile_pool(name="kvp", bufs=1)
    kT = kvp.tile([128, H_LOC, L], F32R)        # [hd, head, y]
    v_sb = kvp.tile([128, YB, E], F32R)         # [y%128, ychunk, e]
    qtp = tc.alloc_tile_pool(name="qtp", bufs=1)
    qT = qtp.tile([128, H_LOC, L], F32R)        # [hd, head, x]
    rstd_pure = qtp.tile([128, YB], F32)
    rstd_sc = qtp.tile([128, YB], F32)

    # ---- phase 1: qT = Wq^T x^T, latT = Wkd^T x^T, ms = mean(lat^2) ----
    latp = tc.alloc_tile_pool(name="latp", bufs=1)
    latT = latp.tile([128, 2, L], F32R)         # [r%128, r//128, y]
    ms_sb = latp.tile([1, L], F32R)

    psA = tc.alloc_tile_pool(name="psA", bufs=1, space="PSUM")

    for xc in range(XC):
        sl = slice(xc * 512, (xc + 1) * 512)
        xts = []
        for kc in range(KC):
            xt = xtp.tile([128, 512], F32R, tag="xtk", name=f"xt{xc}_{kc}")
            nc.sync.dma_start(
                out=xt,
                in_=xT[kc * 128:(kc + 1) * 128,
                       b * L + xc * 512:b * L + (xc + 1) * 512])
            xts.append(xt)
        ms_ps = psA.tile([1, 512], F32, tag="ms", bufs=1)
        for oc in range(2):  # q head chunks
            ps = psA.tile([128, 512], F32, tag=f"q{oc}", bufs=2 - oc, name=f"q{oc}")
            for kc in range(KC):
                nc.tensor.matmul(ps, lhsT=wq_sb[:, kc, oc * 128:(oc + 1) * 128],
                                 rhs=xts[kc], start=(kc == 0), stop=(kc == KC - 1))
            nc.vector.tensor_copy(qT[:, oc, sl], ps)
        for rc in range(2):  # latent chunks
            ps = psA.tile([128, 512], F32, tag=f"lt{rc}", bufs=2, name=f"lt{rc}")
            for kc in range(KC):
                nc.tensor.matmul(ps, lhsT=wkd_sb[:, kc, rc * 128:(rc + 1) * 128],
                                 rhs=xts[kc], start=(kc == 0), stop=(kc == KC - 1))
            nc.vector.tensor_copy(latT[:, rc, sl], ps)
            sq = xtp.tile([128, 512], F32R, tag="sq", bufs=2)
            nc.scalar.activation(sq, ps, AF.Square, bias=zero_b)
            nc.tensor.matmul(ms_ps, lhsT=ones_inv, rhs=sq,
                             start=(rc == 0), stop=(rc == 1), skip_group_check=True)
        nc.scalar.copy(ms_sb[:, sl], ms_ps)

    # rstd: transpose ms [1, L] -> msT [128, YB] via K=1 outer-product matmuls
    msT_ps = psA.tile([128, YB], F32, tag="ms", bufs=1)
    for j in range(YB):
        nc.tensor.matmul(msT_ps[:, j:j + 1],
                         lhsT=ms_sb[:, j * 128:(j + 1) * 128].bitcast(F32),
                         rhs=one33[0:1].bitcast(F32), start=True, stop=True,
                         skip_group_check=True)
    t_p = xtp.tile([128, YB], F32, tag="tp", bufs=1)
    nc.scalar.activation(t_p, msT_ps, AF.Sqrt, bias=eps_b, scale=1.0)
    nc.vector.reciprocal(rstd_pure, t_p)
    t_s = xtp.tile([128, YB], F32, tag="ts", bufs=1)
    nc.scalar.activation(t_s, msT_ps, AF.Sqrt, bias=epsh_b, scale=float(HD))
    nc.vector.reciprocal(rstd_sc, t_s)

    # ---- phase 2: kT = Wuk^T latT, v = lat @ Wuv (rstd folded into v) ----
    for yc2 in range(XC):  # 512-wide y chunks
        sl = slice(yc2 * 512, (yc2 + 1) * 512)
        for ec in range(H_LOC):
            ps = psA.tile([128, 512], F32, tag=f"q{ec}", bufs=2 - ec, name=f"kv{ec}")
            for rc in range(2):
                nc.tensor.matmul(ps, lhsT=wuk_sb[:, rc, ec * 128:(ec + 1) * 128],
                                 rhs=latT[:, rc, sl], start=(rc == 0), stop=(rc == 1))
            nc.scalar.copy(kT[:, ec, sl], ps)
        for j in range(4):
            yg = yc2 * 4 + j
            ps = psA.tile([128, E], F32, tag="lt0", bufs=2, name="vps")
            for rc in range(2):
                nc.tensor.matmul(ps, lhsT=latT[:, rc, yg * 128:(yg + 1) * 128],
                                 rhs=wuv_sb[:, rc, :], start=(rc == 0), stop=(rc == 1))
            nc.vector.tensor_scalar_mul(v_sb[:, yg, :], ps, rstd_pure[:, yg:yg + 1])

    psA.release()
    latp.release()

    # ---- attention + o_proj ----
    workp = tc.alloc_tile_pool(name="workp", bufs=3)
    psC = tc.alloc_tile_pool(name="psC", bufs=1, space="PSUM")

    for xb in range(XB):
        xsl = slice(xb * 512, (xb + 1) * 512)
        nyc = 4 * xb + 4               # causal: y chunks needed
        ao_ps = [psC.tile([128, 512], F32, tag=f"ao{h}", bufs=1, name=f"ao{h}")
                 for h in range(H_LOC)]
        l_ps = psC.tile([1, 2 * 512], F32, tag="lmix", bufs=1)
        for iy in range(nyc):
            st = psC.tile([128, 2 * 512], F32, tag="st", bufs=1)
            at = workp.tile([128, 2 * 512], F32R, tag="at", bufs=6)
            for h in range(H_LOC):
                nc.tensor.matmul(st[:, h * 512:(h + 1) * 512],
                                 lhsT=kT[:, h, iy * 128:(iy + 1) * 128],
                                 rhs=qT[:, h, xsl], start=True, stop=True,
                                 skip_group_check=True)
            nc.scalar.activation(at, st, AF.Exp, bias=zero_b,
                                 scale=rstd_sc[:, iy:iy + 1])
            if iy >= 4 * xb:           # diagonal chunk: zero out y > x
                off = (iy - 4 * xb) * 128
                for h in range(H_LOC):
                    nc.gpsimd.affine_select(
                        out=at[:, h * 512:(h + 1) * 512],
                        in_=at[:, h * 512:(h + 1) * 512],
                        compare_op=ALU.is_ge, fill=0.0, base=-off,
                        pattern=[[1, 512]], channel_multiplier=-1)
            for h in range(H_LOC):
                nc.tensor.matmul(ao_ps[h],
                                 lhsT=v_sb[:, iy, h * 128:(h + 1) * 128],
                                 rhs=at[:, h * 512:(h + 1) * 512],
                                 start=(iy == 0), stop=(iy == nyc - 1),
                                 skip_group_check=True)
                nc.tensor.matmul(l_ps[0:1, h * 512:(h + 1) * 512],
                                 lhsT=ones1, rhs=at[:, h * 512:(h + 1) * 512],
                                 start=(iy == 0), stop=(iy == nyc - 1),
                                 skip_group_check=True)
        ao_sb = workp.tile([128, H_LOC, 512], F32R, tag="ao_sb", bufs=2)
        for h in range(H_LOC):
            nc.vector.tensor_copy(ao_sb[:, h, :], ao_ps[h])
        l_sb = workp.tile([1, 2 * 512], F32R, tag="l_sb", bufs=2)
        nc.scalar.copy(l_sb, l_ps)
        lT_ps = psC.tile([128, 2 * 4], F32, tag="lmix", bufs=1)
        for h in range(H_LOC):
            for xs in range(4):
                nc.tensor.matmul(
                    lT_ps[:, h * 4 + xs:h * 4 + xs + 1],
                    lhsT=l_sb[0:1, h * 512 + xs * 128:h * 512 + (xs + 1) * 128]
                    .bitcast(F32),
                    rhs=one33[0:1].bitcast(F32), start=True, stop=True,
                    skip_group_check=True)
        recip_l = workp.tile([128, 2 * 4], F32, tag="recip", bufs=2)
        nc.vector.reciprocal(recip_l, lT_ps)

        for xs in range(4):
            r0 = b * L + xb * 512 + xs * 128
            for mc in range(4):
                msl = slice(mc * 512, (mc + 1) * 512)
                wps = [psC.tile([128, 512], F32, tag=f"w{h}", bufs=1, name=f"w{h}")
                       for h in range(H_LOC)]
                for h in range(H_LOC):
                    nc.tensor.matmul(wps[h],
                                     lhsT=ao_sb[:, h, xs * 128:(xs + 1) * 128],
                                     rhs=wo_sb[:, h, msl], start=True, stop=True,
                                     skip_group_check=True)
                t = workp.tile([128, 512], F32, tag="wt")
                nc.scalar.activation(t, wps[0], AF.Copy, scale=recip_l[:, xs:xs + 1])
                o = workp.tile([128, 512], F32, tag="o")
                nc.vector.scalar_tensor_tensor(
                    out=o, in0=wps[1], scalar=recip_l[:, 4 + xs:4 + xs + 1],
                    in1=t, op0=ALU.mult, op1=ALU.add)
                nc.gpsimd.dma_start(out=out_d[r0:r0 + 128, msl], in_=o)

    psC.release()
    workp.release()
    qtp.release()
    kvp.release()
    xtp.release()


def _emit(tc):
    nc = tc.nc
    env = {}
    env["xT"] = nc.dram_tensor("xT", [D, BL], F32R, kind="ExternalInput").ap()
    wq_d = nc.dram_tensor("wq", [D, DQ], F32R, kind="ExternalInput").ap()
    wkd_d = nc.dram_tensor("wkd", [D, R], F32R, kind="ExternalInput").ap()
    wuk_d = nc.dram_tensor("wuk", [R, DQ], F32R, kind="ExternalInput").ap()
    wuv_d = nc.dram_tensor("wuv", [R, E], F32R, kind="ExternalInput").ap()
    wo_d = nc.dram_tensor("wo", [E, D], F32R, kind="ExternalInput").ap()
    env["out_d"] = nc.dram_tensor("out", [BL, D], F32, kind="ExternalOutput").ap()

    constp = tc.alloc_tile_pool(name="constp", bufs=1)
    tmp1 = constp.tile([128, 1], F32)
    nc.vector.memset(tmp1, 1.0)
    tmp2 = constp.tile([128, 1], F32)
    nc.vector.memset(tmp2, 1.0 / R)
    env["ones1"] = constp.tile([128, 1], F32R, name="ones1")
    nc.scalar.copy(env["ones1"], tmp1)
    env["ones_inv"] = constp.tile([128, 1], F32R, name="ones_inv")
    nc.scalar.copy(env["ones_inv"], tmp2)
    tmp3 = constp.tile([33, 1], F32)
    nc.vector.memset(tmp3, 1.0)
    env["one33"] = constp.tile([33, 1], F32R, name="one33")
    nc.scalar.copy(env["one33"], tmp3)
    env["zero_b"] = constp.tile([128, 1], F32, name="zero_b")
    nc.vector.memset(env["zero_b"], 0.0)
    env["eps_b"] = constp.tile([128, 1], F32, name="eps_b")
    nc.vector.memset(env["eps_b"], EPS)
    env["epsh_b"] = constp.tile([128, 1], F32, name="epsh_b")
    nc.vector.memset(env["epsh_b"], HD * EPS)

    wp = tc.alloc_tile_pool(name="wp", bufs=1)
    env["wq_sb"] = wq_sb = wp.tile([128, KC, DQ], F32R, name="wq_sb")
    env["wkd_sb"] = wkd_sb = wp.tile([128, KC, R], F32R, name="wkd_sb")
    for kc in range(KC):
        nc.scalar.dma_start(out=wq_sb[:, kc, :], in_=wq_d[kc * 128:(kc + 1) * 128, :])
        nc.scalar.dma_start(out=wkd_sb[:, kc, :], in_=wkd_d[kc * 128:(kc + 1) * 128, :])
    env["wuk_sb"] = wuk_sb = wp.tile([128, 2, DQ], F32R, name="wuk_sb")
    env["wuv_sb"] = wuv_sb = wp.tile([128, 2, E], F32R, name="wuv_sb")
    nc.scalar.dma_start(out=wuk_sb, in_=wuk_d.rearrange("(c p) n -> p c n", p=128))
    nc.scalar.dma_start(out=wuv_sb, in_=wuv_d.rearrange("(c p) n -> p c n", p=128))
    env["wo_sb"] = wo_sb = wp.tile([128, H_LOC, D], F32R, name="wo_sb")
    for kc in range(H_LOC):
        nc.scalar.dma_start(out=wo_sb[:, kc, :], in_=wo_d[kc * 128:(kc + 1) * 128, :])

    for b in range(B):
        _emit_batch(tc, b, env)

    wp.release()
    constp.release()


_NC_CACHE = None


def _build():
    global _NC_CACHE
    if _NC_CACHE is None:
        nc = bacc.Bacc()
        with tile.TileContext(nc) as tc:
            _emit(tc)
        nc.compile()
        _NC_CACHE = nc
    return _NC_CACHE


def make_in_maps(inputs):
    x = np.asarray(inputs["x"], dtype=np.float32)
    xT = np.ascontiguousarray(x.reshape(BL, D).T)
    Wq = np.asarray(inputs["Wq"], dtype=np.float32)
    Wkd = np.ascontiguousarray(np.asarray(inputs["Wkv_down"], dtype=np.float32))
    Wup = np.asarray(inputs["Wkv_up"], dtype=np.float32) * np.asarray(
        inputs["kv_norm_w"], dtype=np.float32)[:, None]
    Wo = np.asarray(inputs["Wo"], dtype=np.float32)

    in_maps = []
    for c in range(NC_CORES):
        in_maps.append({
            "xT": xT,
            "wq": np.ascontiguousarray(Wq[:, c * DQ:(c + 1) * DQ]),
            "wkd": Wkd,
            "wuk": np.ascontiguousarray(Wup[:, c * DQ:(c + 1) * DQ]),
            "wuv": np.ascontiguousarray(
                Wup[:, HEADS * HD + c * E:HEADS * HD + (c + 1) * E]),
            "wo": np.ascontiguousarray(Wo[c * E:(c + 1) * E, :]),
        })
    return in_maps


def kernel(x, Wq, Wkv_down, kv_norm_w, Wkv_up, Wo):
    in_maps = make_in_maps(dict(x=x, Wq=Wq, Wkv_down=Wkv_down,
                                kv_norm_w=kv_norm_w, Wkv_up=Wkv_up, Wo=Wo))
    nc = _build()
    res = run_bass_kernel_spmd(nc, in_maps, core_ids=list(range(NC_CORES)))
    acc = res.results[0]["out"].astype(np.float32)
    for r in res.results[1:]:
        acc = acc + r["out"]
    return acc.reshape(B, L, D)
